# revision 3
# baseline (speedup 1.0000x reference)
"""Single-head causal attention (B=4, S=4096, D=1024, H=64) on 8 trn2 cores.

Sharding: core c -> batch b = c % 4, role r = c // 4.  Per batch, global
q-tiles (512 rows) interleave: role 0 owns {0,2,4,6}, role 1 owns {1,3,5,7}.
Core loads only its OWN 2048 rows of x, projects Q/K/V, exchanges K/V per
512-row chunk via AllGather; all K/V (incl own) round-trips the collective so
SBUF addressing stays uniform across cores.

Attention tile i runs 2i+2 k-chunk slots over global chunks 0..2i+1.  The
last two slots carry host masks (maskA: tri|ones, maskB: zeros|tri by role).
Scores and unmasked PV run as fp8e4 DoubleRow matmuls; masked slots run bf16
so the mask multiply gets the DVE 2x mode.  DoubleRow pair-1 of the K/Q
weights points into a shared zeroed 512-col block via the AP pair stride, so
only one small memset is needed instead of zero-filling every pad region.
Scores are computed transposed (sT[k,q]) so exp output feeds PV directly;
softmax max-subtraction is skipped (|score| small) and the denominator is a
ones-column appended to V.
"""

import math

import ml_dtypes
import numpy as np

B, S, D, H = 4, 4096, 1024, 64
NT = 4           # local q-tiles per core (512 rows each)
QT = 512
KC = 512
NKB = 4          # 128-row k-blocks per chunk
NCHUNK = S // KC
G = H + 1        # V group width (ones column appended)
VG = 80          # padded V group (DoubleRow weight pair step must be %16==0)
KPKT = 64 * KC                   # K^T bytes per chunk packet (fp8 [64,512])
VPKT = 128 * NKB * G * 2         # V bytes per chunk packet (bf16 [128,(4,65)])
PKT = KPKT + VPKT

_compiled = None
TRACE = False
LAST_RESULT = None


def _build():
    import concourse.bass as bass
    import concourse.mybir as mybir
    from concourse import bacc
    from concourse.masks import make_identity
    from concourse.tile import TileContext

    fp32 = mybir.dt.float32
    bf16 = mybir.dt.bfloat16
    f8 = mybir.dt.float8e4
    u8 = mybir.dt.uint8
    AF = mybir.ActivationFunctionType
    DR = mybir.MatmulPerfMode.DoubleRow

    nc = bacc.Bacc(None, target_bir_lowering=False)
    x_kv = nc.dram_tensor("x_kv", [NT * KC, D], fp32, kind="ExternalInput")
    wqk_d = nc.dram_tensor("wqk", [128, 8 * 128], bf16, kind="ExternalInput")
    wv_d = nc.dram_tensor("wv", [128, 8 * H], bf16, kind="ExternalInput")
    bqk_d = nc.dram_tensor("bqk", [128, 1], fp32, kind="ExternalInput")
    bv_d = nc.dram_tensor("bv", [128, NKB * H], fp32, kind="ExternalInput")
    zeros_d = nc.dram_tensor("zeros", [128, 8 * KC], u8, kind="ExternalInput")
    idbf_d = nc.dram_tensor("idbf", [128, 128], bf16, kind="ExternalInput")
    idf32_d = nc.dram_tensor("idf32", [128, 128], fp32, kind="ExternalInput")
    maskA_d = nc.dram_tensor("maskA", [128, 4 * KC], bf16, kind="ExternalInput")
    maskB_d = nc.dram_tensor("maskB", [128, 4 * KC], bf16, kind="ExternalInput")
    y_d = nc.dram_tensor("y", [NT * QT, H], fp32, kind="ExternalOutput")
    kv_out = nc.dram_tensor("kv_out", [NT, PKT], u8)
    kv_alls = [nc.dram_tensor(f"kv_all{c}", [2, PKT], u8) for c in range(NT)]

    with TileContext(nc) as tc:
        with (
            tc.tile_pool(name="const", bufs=1) as cpool,
            tc.tile_pool(name="xstage", bufs=8) as xpool,
            tc.tile_pool(name="stg", bufs=2) as spool,
            tc.tile_pool(name="pX", bufs=4) as xppool,
            tc.tile_pool(name="pXb", bufs=3) as xbpool,
            tc.tile_pool(name="fin", bufs=2) as fpool,
            tc.tile_pool(name="prj", bufs=3, space="PSUM") as prj,    # tp/proj/fin ring
            tc.tile_pool(name="psS", bufs=2, space="PSUM") as psS,    # scores
            tc.tile_pool(name="psO", bufs=1, space="PSUM") as psO,    # out accum
        ):
            # ---------------- persistent SBUF ----------------
            wqk = cpool.tile([128, 8 * 128], bf16, tag="wqk")
            wv = cpool.tile([128, 8 * H], bf16, tag="wv")
            bqk = cpool.tile([128, 1], fp32, tag="bqk")
            bv = cpool.tile([128, NKB * H], fp32, tag="bv")
            bv_v = cpool.tile([128, NKB * H], fp32, tag="bvv")
            maskA = cpool.tile([128, 4 * KC], bf16, tag="maskA")
            maskB = cpool.tile([128, 4 * KC], bf16, tag="maskB")
            id_bf = cpool.tile([128, 128], bf16, tag="idbf")
            id_f32 = cpool.tile([128, 128], fp32, tag="idf32")
            # K^T / Q^T fp8 DoubleRow layout: interleaved (data, zero) 512-col
            # pairs per chunk|tile, so the pair stride is uniformly KC.  h on
            # partitions 0:64; 64:128 of data cols zeroed per chunk on Pool.
            KT8 = cpool.tile([128, NCHUNK * 2 * KC], f8, tag="KT8")
            QT8 = cpool.tile([128, NT * 2 * KC], f8, tag="QT8")
            # V (+ones col) per global chunk: bf16 [128,(chunk,kb,65)], fp8 padded to 80
            Vtb = cpool.tile([128, NCHUNK * NKB * G], bf16, tag="Vtb")
            Vt8 = cpool.tile([128, NCHUNK * NKB * VG], f8, tag="Vt8")

            zeros = zeros_d

            # x loads first so chunk-0 transposes start ASAP; weights next.
            x_pieces = [[None] * 4 for _ in range(NT)]

            def load_x(c, fine=False):
                if fine:
                    for p in range(4):
                        xp = xpool.tile([128, D], bf16, tag="xf")
                        nc.gpsimd.dma_start(
                            out=xp[:],
                            in_=x_kv[c * KC + p * 128: c * KC + (p + 1) * 128, :]
                                  .rearrange("(t p) d -> (p t) d", p=128),
                        )
                        x_pieces[c][p] = xp[:]
                    return
                for hh in range(2):
                    xp = xpool.tile([128, 2 * D], bf16, tag="xs")
                    nc.gpsimd.dma_start(
                        out=xp.rearrange("p (t d) -> p t d", d=D),
                        in_=x_kv[c * KC + hh * 256: c * KC + (hh + 1) * 256, :]
                              .rearrange("(t p) d -> p t d", p=128),
                    )
                    x_pieces[c][2 * hh] = xp.rearrange("p (t d) -> p t d", d=D)[:, 0]
                    x_pieces[c][2 * hh + 1] = xp.rearrange("p (t d) -> p t d", d=D)[:, 1]

            load_x(0, fine=True)
            nc.sync.dma_start(out=id_bf[:], in_=idbf_d[:])
            nc.sync.dma_start(out=wqk[:], in_=wqk_d[:])
            nc.sync.dma_start(out=bqk[:], in_=bqk_d[:])
            # zero the DoubleRow pads of K/Q on Pool, one chunk-pair of cols
            # at a time so the gpsimd queue never backs up ahead of gathers
            def kq_zero(buf, g):
                nc.gpsimd.memset(buf[:, 2 * g * KC:(2 * g + 2) * KC], 0.0)
            kq_zero(QT8, 0)
            kq_zero(KT8, 0)
            kq_zero(KT8, 1)
            load_x(1)
            nc.sync.dma_start(out=wv[:], in_=wv_d[:])
            nc.sync.dma_start(out=bv[:], in_=bv_d[:])
            nc.sync.dma_start(out=id_f32[:], in_=idf32_d[:])
            nc.vector.tensor_copy(bv_v[:], bv[:])

            # zero pads: V pad cols + ones cols (K/Q pads zeroed per chunk)
            v8_grp = Vt8.rearrange("p (n s) -> p n s", s=VG)
            nc.vector.memset(v8_grp[:, :, H:VG], 0.0)
            nc.vector.memset(v8_grp[:, :, H:G], 1.0)
            vb_grp = Vtb.rearrange("p (n s) -> p n s", s=G)
            nc.vector.memset(vb_grp[:, :, H:G], 1.0)

            # ---------------- projection of local chunk c ----------------
            def project_chunk(c):
                xT = spool.tile([128, 8 * KC], bf16, tag="xT")  # (db, row)
                for p in range(4):  # 128-row piece
                    tp = prj.tile([128, 1024], bf16, tag="prj")
                    for db in range(8):
                        nc.tensor.transpose(
                            tp[:, db * 128:(db + 1) * 128],
                            x_pieces[c][p][:, db * 128:(db + 1) * 128],
                            id_bf[:],
                        )
                    nc.vector.tensor_copy(
                        xT.rearrange("q (db r) -> q db r", r=KC)[:, :, p * 128:(p + 1) * 128],
                        tp.rearrange("q (db r) -> q db r", r=128),
                    )
                # stacked Q|K projection: out partitions 0:64 Q^T, 64:128 K^T
                ps_qk_t = prj.tile([128, KC], fp32, tag="prj")
                ps_qk = ps_qk_t[:]
                for db in range(8):
                    nc.tensor.matmul(
                        ps_qk[:],
                        wqk[:, db * 128:(db + 1) * 128],
                        xT[:, db * KC:(db + 1) * KC],
                        start=(db == 0), stop=(db == 7),
                    )
                # evac: h 0:64 on partitions 0:64 of the data cols
                kt_st = spool.tile([64, KC], f8, tag="kst")
                nc.vector.tensor_scalar_add(
                    kt_st[:], ps_qk[64:128, :], bqk[64:128, :])
                nc.vector.tensor_scalar_add(
                    QT8[0:64, 2 * c * KC:(2 * c + 1) * KC], ps_qk[0:64, :], bqk[0:64, :])

                # V projection (direct [k,h] layout), all 4 k-blocks in one bank
                ps_v_t = prj.tile([128, NKB * H], fp32, tag="prj")
                ps_v = ps_v_t[:]
                for kb in range(NKB):
                    for db in range(8):
                        nc.tensor.matmul(
                            ps_v[:, kb * H:(kb + 1) * H],
                            xT[:, db * KC + kb * 128: db * KC + (kb + 1) * 128],
                            wv[:, db * H:(db + 1) * H],
                            start=(db == 0), stop=(db == 7),
                        )
                vt_st = spool.tile([128, NKB * G], bf16, tag="vst")
                nc.vector.memset(
                    vt_st.rearrange("p (n s) -> p n s", s=G)[:, :, H:G], 1.0)
                nc.vector.tensor_add(
                    vt_st.rearrange("p (n s) -> p n s", s=G)[:, :, 0:H],
                    ps_v[:], bv_v[:],
                )
                return kt_st, vt_st

            def exchange_chunk(c, kt_st, vt_st):
                nc.sync.dma_start(
                    out=kv_out[c:c + 1, 0:KPKT].rearrange("o (h s) -> (o h) s", s=KC),
                    in_=kt_st.bitcast(u8),
                )
                nc.sync.dma_start(
                    out=kv_out[c:c + 1, KPKT:].rearrange("o (k g) -> (o k) g", g=NKB * G * 2),
                    in_=vt_st.bitcast(u8),
                )
                nc.gpsimd.collective_compute(
                    "AllGather",
                    mybir.AluOpType.bypass,
                    replica_groups=[[0, 4], [1, 5], [2, 6], [3, 7]],
                    ins=[kv_out[c:c + 1, :]],
                    outs=[kv_alls[c][:]],
                )
            def unpack_chunk(c):
                for r in range(2):
                    g = 2 * c + r  # global chunk index
                    nc.sync.dma_start(
                        out=KT8[0:64, 2 * g * KC:(2 * g + 1) * KC].bitcast(u8),
                        in_=kv_alls[c][r, 0:KPKT].rearrange("(h s) -> h s", s=KC),
                    )
                    nc.sync.dma_start(
                        out=Vtb[:, g * NKB * G:(g + 1) * NKB * G].bitcast(u8),
                        in_=kv_alls[c][r, KPKT:].rearrange("(k g) -> k g", g=NKB * G * 2),
                    )

            def vt8_copies(c):
                # deferred off the collective path so gathers flow back-to-back
                for r in range(2):
                    g = 2 * c + r
                    nc.gpsimd.tensor_copy(
                        Vt8.rearrange("p (n s) -> p n s", s=VG)
                           [:, g * NKB:(g + 1) * NKB, 0:G],
                        Vtb.rearrange("p (n s) -> p n s", s=G)
                           [:, g * NKB:(g + 1) * NKB, :],
                    )

            # ---------------- attention tile i ----------------
            KT8v = KT8.rearrange("p (g two s) -> p g two s", two=2, s=KC)
            QT8v = QT8.rearrange("p (g two s) -> p g two s", two=2, s=KC)

            def attention_slots(i):
                nslot = 2 * i + 2
                oT = psO.tile([128, QT], fp32, tag="oT")
                for j in range(nslot):
                    masked = j >= nslot - 2
                    if masked:
                        pX = xbpool.tile([128, NKB * KC], bf16, tag="pXb")
                    else:
                        pX = xppool.tile([128, NKB * KC], f8, tag="pX8")
                    for h2 in range(2):
                        sT = psS.tile([128, 2 * KC], fp32, tag="sT")
                        for kk in range(2):
                            kb = 2 * h2 + kk
                            nc.tensor.matmul(
                                sT[:, kk * KC:(kk + 1) * KC],
                                KT8v[:, j, :, kb * 128:(kb + 1) * 128],
                                QT8v[:, i],
                                start=True, stop=True, perf_mode=DR,
                            )
                        nc.scalar.activation(
                            pX[:, h2 * 2 * KC:(h2 + 1) * 2 * KC], sT[:], AF.Exp,
                            scale=1.0 / math.sqrt(H),
                        )
                        if masked:
                            mk = maskA if j == nslot - 2 else maskB
                            nc.vector.tensor_mul(
                                pX[:, h2 * 2 * KC:(h2 + 1) * 2 * KC],
                                pX[:, h2 * 2 * KC:(h2 + 1) * 2 * KC],
                                mk[:, h2 * 2 * KC:(h2 + 1) * 2 * KC],
                            )
                            for kk in range(2):
                                kb = 2 * h2 + kk
                                nc.tensor.matmul(
                                    oT[0:G, :],
                                    Vtb[:, (j * NKB + kb) * G:(j * NKB + kb + 1) * G],
                                    pX[:, kb * KC:(kb + 1) * KC],
                                    start=(j == 0 and kb == 0),
                                    stop=(j == nslot - 1 and kb == NKB - 1),
                                    skip_group_check=True,
                                )
                        else:
                            nc.tensor.matmul(
                                oT[0:VG, :],
                                Vt8[:, (j * NKB + 2 * h2) * VG:
                                       (j * NKB + 2 * h2 + 2) * VG]
                                    .rearrange("p (two g) -> p two g", two=2),
                                pX[:, h2 * 2 * KC:(h2 + 1) * 2 * KC]
                                    .rearrange("p (two s) -> p two s", two=2),
                                start=(j == 0 and h2 == 0),
                                stop=False, perf_mode=DR,
                                skip_group_check=True,
                            )
                return oT

            def attention_finA(i, oT):
                oT_sb = fpool.tile([128, QT], fp32, tag="oTsb")
                nc.vector.tensor_copy(oT_sb[0:G, :], oT[0:G, :])
                return oT_sb

            def attention_finB(i, oT_sb):
                po_t = prj.tile([128, 1024], bf16, tag="prj")
                po = po_t.bitcast(fp32)
                for t in range(NKB):
                    nc.tensor.transpose(
                        po[:, t * G:(t + 1) * G],
                        oT_sb[0:G, t * 128:(t + 1) * 128], id_f32[0:G, 0:G],
                    )
                rec = fpool.tile([128, NKB], fp32, tag="rec")
                nc.vector.reciprocal(
                    rec[:], po[:, 0:NKB * G].rearrange("p (t s) -> p t s", s=G)[:, :, H:G])
                y_sb = fpool.tile([128, NKB * H], fp32, tag="ysb")
                for t in range(NKB):
                    nc.vector.tensor_scalar_mul(
                        y_sb[:, t * H:(t + 1) * H], po[:, t * G: t * G + H],
                        rec[:, t:t + 1],
                    )
                    nc.sync.dma_start(
                        out=y_d[i * QT + t * 128: i * QT + (t + 1) * 128, :],
                        in_=y_sb[:, t * H:(t + 1) * H],
                    )

            # ---------------- schedule ----------------
            st0 = project_chunk(0); exchange_chunk(0, *st0)
            unpack_chunk(0)
            nc.sync.dma_start(out=maskA[:], in_=maskA_d[:])
            kq_zero(KT8, 2); kq_zero(KT8, 3); kq_zero(QT8, 1)
            st1 = project_chunk(1); exchange_chunk(1, *st1)
            unpack_chunk(1)
            nc.sync.dma_start(out=maskB[:], in_=maskB_d[:])
            load_x(2)
            kq_zero(KT8, 4); kq_zero(KT8, 5); kq_zero(QT8, 2)
            vt8_copies(0)
            ob0 = attention_finA(0, attention_slots(0))
            st2 = project_chunk(2); exchange_chunk(2, *st2)
            unpack_chunk(2)
            load_x(3)
            kq_zero(KT8, 6); kq_zero(KT8, 7); kq_zero(QT8, 3)
            vt8_copies(1)
            ob1 = attention_finA(1, attention_slots(1))
            attention_finB(0, ob0)
            st3 = project_chunk(3); exchange_chunk(3, *st3)
            unpack_chunk(3)
            vt8_copies(2)
            ob2 = attention_finA(2, attention_slots(2))
            attention_finB(1, ob1)
            vt8_copies(3)
            ob3 = attention_finA(3, attention_slots(3))
            attention_finB(2, ob2)
            attention_finB(3, ob3)

    nc.compile()
    return nc


def _masks_for(role: int):
    # tri[kb][p, f] = 1.0 where f >= kb*128 + p  (keep q >= k in diag chunk)
    p = np.arange(128)[:, None]
    f = np.arange(KC)[None, :]
    tri = np.concatenate(
        [(f >= kb * 128 + p).astype(np.float32) for kb in range(NKB)], axis=1
    )
    ones = np.ones((128, NKB * KC), dtype=np.float32)
    zero = np.zeros((128, NKB * KC), dtype=np.float32)
    maskA = tri if role == 0 else ones
    maskB = zero if role == 0 else tri
    return (np.ascontiguousarray(maskA).astype(ml_dtypes.bfloat16),
            np.ascontiguousarray(maskB).astype(ml_dtypes.bfloat16))


def kernel(x, Wq_w, Wq_b, Wk_w, Wk_b, Wv_w, Wv_b):
    global _compiled, LAST_RESULT
    from concourse.bass_utils import run_bass_kernel_spmd

    x = np.asarray(x, dtype=np.float32)
    wqk_dm = np.concatenate([np.asarray(Wq_w), np.asarray(Wk_w)], axis=1)
    wqk = np.ascontiguousarray(
        wqk_dm.reshape(8, 128, 128).transpose(1, 0, 2).reshape(128, 8 * 128)
    ).astype(ml_dtypes.bfloat16)
    bqk = np.concatenate([np.asarray(Wq_b), np.asarray(Wk_b)])[:, None].astype(np.float32)
    wv = np.ascontiguousarray(
        np.asarray(Wv_w, dtype=np.float32).reshape(8, 128, H)
        .transpose(1, 0, 2).reshape(128, 8 * H)
    ).astype(ml_dtypes.bfloat16)
    bv = np.ascontiguousarray(np.tile(
        np.broadcast_to(np.asarray(Wv_b, dtype=np.float32)[None, :], (128, H)),
        (1, NKB)))

    if _compiled is None:
        _compiled = _build()
    nc = _compiled

    in_maps = []
    for c in range(8):
        b, role = c % 4, c // 4
        mA, mB = _masks_for(role)
        x_own = np.ascontiguousarray(
            x[b].reshape(NCHUNK, KC, D)[role::2].reshape(NT * KC, D)
        )
        in_maps.append({
            "x_kv": x_own,
            "wqk": wqk, "wv": wv, "bqk": bqk, "bv": bv,
            "maskA": mA, "maskB": mB,
            "zeros": np.zeros((128, 8 * KC), dtype=np.uint8),
            "idbf": np.eye(128, dtype=ml_dtypes.bfloat16),
            "idf32": np.eye(128, dtype=np.float32),
        })
    kw = {}
    if TRACE:
        kw = dict(trace=True, trace_cores=list(range(8)))
    res = run_bass_kernel_spmd(nc, in_maps, core_ids=list(range(8)), **kw)
    LAST_RESULT = res

    out = np.empty((B, S, H), dtype=np.float32)
    for c in range(8):
        b, role = c % 4, c // 4
        y = res.results[c]["y"]
        for i in range(NT):
            g = 2 * i + role
            out[b, g * QT:(g + 1) * QT, :] = y[i * QT:(i + 1) * QT, :]
    return out


# revision 4
# speedup vs baseline: 1.0186x; 1.0186x over previous
"""Single-head causal attention (B=4, S=4096, D=1024, H=64) on 8 trn2 cores.

Sharding: core c -> batch b = c % 4, role r = c // 4.  Per batch, global
q-tiles (512 rows) interleave: role 0 owns {0,2,4,6}, role 1 owns {1,3,5,7}.
Core loads only its OWN 2048 rows of x, projects Q/K/V, exchanges K/V per
512-row chunk via AllGather; all K/V (incl own) round-trips the collective so
SBUF addressing stays uniform across cores.

Attention tile i runs 2i+2 k-chunk slots over global chunks 0..2i+1.  The
last two slots carry host masks (maskA: tri|ones, maskB: zeros|tri by role).
Scores and unmasked PV run as fp8e4 DoubleRow matmuls; masked slots run bf16
so the mask multiply gets the DVE 2x mode.  DoubleRow pair-1 of the K/Q
weights points into a shared zeroed 512-col block via the AP pair stride, so
only one small memset is needed instead of zero-filling every pad region.
Scores are computed transposed (sT[k,q]) so exp output feeds PV directly;
softmax max-subtraction is skipped (|score| small) and the denominator is a
ones-column appended to V.
"""

import math

import ml_dtypes
import numpy as np

B, S, D, H = 4, 4096, 1024, 64
NT = 4           # local q-tiles per core (512 rows each)
QT = 512
KC = 512
NKB = 4          # 128-row k-blocks per chunk
NCHUNK = S // KC
G = H + 1        # V group width (ones column appended)
VG = 80          # padded V group (DoubleRow weight pair step must be %16==0)
KPKT = 64 * KC                   # K^T bytes per chunk packet (fp8 [64,512])
VPKT = 128 * NKB * G * 2         # V bytes per chunk packet (bf16 [128,(4,65)])
PKT = KPKT + VPKT

_compiled = None
TRACE = False
LAST_RESULT = None


def _build():
    import concourse.bass as bass
    import concourse.mybir as mybir
    from concourse import bacc
    from concourse.masks import make_identity
    from concourse.tile import TileContext

    fp32 = mybir.dt.float32
    bf16 = mybir.dt.bfloat16
    f8 = mybir.dt.float8e4
    u8 = mybir.dt.uint8
    AF = mybir.ActivationFunctionType
    DR = mybir.MatmulPerfMode.DoubleRow

    nc = bacc.Bacc(None, target_bir_lowering=False)
    x_kv = nc.dram_tensor("x_kv", [NT * KC, D], fp32, kind="ExternalInput")
    wqk_d = nc.dram_tensor("wqk", [128, 8 * 128], bf16, kind="ExternalInput")
    wv_d = nc.dram_tensor("wv", [128, 8 * H], bf16, kind="ExternalInput")
    bqk_d = nc.dram_tensor("bqk", [128, 1], fp32, kind="ExternalInput")
    bv_d = nc.dram_tensor("bv", [128, NKB * H], fp32, kind="ExternalInput")
    zeros_d = nc.dram_tensor("zeros", [128, 8 * KC], u8, kind="ExternalInput")
    idbf_d = nc.dram_tensor("idbf", [128, 128], bf16, kind="ExternalInput")
    idf32_d = nc.dram_tensor("idf32", [128, 128], fp32, kind="ExternalInput")
    maskA_d = nc.dram_tensor("maskA", [128, 4 * KC], bf16, kind="ExternalInput")
    maskB_d = nc.dram_tensor("maskB", [128, 4 * KC], bf16, kind="ExternalInput")
    y_d = nc.dram_tensor("y", [NT * QT, H], fp32, kind="ExternalOutput")
    kv_out = nc.dram_tensor("kv_out", [NT, PKT], u8)
    kv_alls = [nc.dram_tensor(f"kv_all{c}", [2, PKT], u8) for c in range(NT)]

    with TileContext(nc) as tc:
        with (
            tc.tile_pool(name="const", bufs=1) as cpool,
            tc.tile_pool(name="xstage", bufs=8) as xpool,
            tc.tile_pool(name="stg", bufs=2) as spool,
            tc.tile_pool(name="pX", bufs=4) as xppool,
            tc.tile_pool(name="pXb", bufs=3) as xbpool,
            tc.tile_pool(name="fin", bufs=2) as fpool,
            tc.tile_pool(name="prj", bufs=3, space="PSUM") as prj,    # tp/proj/fin ring
            tc.tile_pool(name="psS", bufs=2, space="PSUM") as psS,    # scores
            tc.tile_pool(name="psO", bufs=1, space="PSUM") as psO,    # out accum
        ):
            # ---------------- persistent SBUF ----------------
            wqk = cpool.tile([128, 8 * 128], bf16, tag="wqk")
            wv = cpool.tile([128, 8 * H], bf16, tag="wv")
            bqk = cpool.tile([128, 1], fp32, tag="bqk")
            bv = cpool.tile([128, NKB * H], fp32, tag="bv")
            bv_v = cpool.tile([128, NKB * H], fp32, tag="bvv")
            maskA = cpool.tile([128, 4 * KC], bf16, tag="maskA")
            maskB = cpool.tile([128, 4 * KC], bf16, tag="maskB")
            id_bf = cpool.tile([128, 128], bf16, tag="idbf")
            id_f32 = cpool.tile([128, 128], fp32, tag="idf32")
            # K^T / Q^T fp8 DoubleRow layout: interleaved (data, zero) 512-col
            # pairs per chunk|tile, so the pair stride is uniformly KC.  h on
            # partitions 0:64; 64:128 of data cols zeroed per chunk on Pool.
            KT8 = cpool.tile([128, NCHUNK * 2 * KC], f8, tag="KT8")
            QT8 = cpool.tile([128, NT * 2 * KC], f8, tag="QT8")
            # V (+ones col) per global chunk: bf16 [128,(chunk,kb,65)], fp8 padded to 80
            Vtb = cpool.tile([128, NCHUNK * NKB * G], bf16, tag="Vtb")
            Vt8 = cpool.tile([128, NCHUNK * NKB * VG], f8, tag="Vt8")

            zeros = zeros_d

            # x loads first so chunk-0 transposes start ASAP; weights next.
            x_pieces = [[None] * 4 for _ in range(NT)]

            def load_x(c, fine=False):
                if fine:
                    for p in range(4):
                        xp = xpool.tile([128, D], bf16, tag="xf")
                        nc.gpsimd.dma_start(
                            out=xp[:],
                            in_=x_kv[c * KC + p * 128: c * KC + (p + 1) * 128, :]
                                  .rearrange("(t p) d -> (p t) d", p=128),
                        )
                        x_pieces[c][p] = xp[:]
                    return
                for hh in range(2):
                    xp = xpool.tile([128, 2 * D], bf16, tag="xs")
                    nc.gpsimd.dma_start(
                        out=xp.rearrange("p (t d) -> p t d", d=D),
                        in_=x_kv[c * KC + hh * 256: c * KC + (hh + 1) * 256, :]
                              .rearrange("(t p) d -> p t d", p=128),
                    )
                    x_pieces[c][2 * hh] = xp.rearrange("p (t d) -> p t d", d=D)[:, 0]
                    x_pieces[c][2 * hh + 1] = xp.rearrange("p (t d) -> p t d", d=D)[:, 1]

            load_x(0, fine=True)
            nc.sync.dma_start(out=id_bf[:], in_=idbf_d[:])
            nc.sync.dma_start(out=wqk[:], in_=wqk_d[:])
            nc.sync.dma_start(out=bqk[:], in_=bqk_d[:])
            # zero the DoubleRow pads of K/Q on Pool, one chunk-pair of cols
            # at a time so the gpsimd queue never backs up ahead of gathers
            def kq_zero(buf, g):
                nc.gpsimd.memset(buf[:, 2 * g * KC:(2 * g + 2) * KC], 0.0)
            kq_zero(QT8, 0)
            kq_zero(KT8, 0)
            kq_zero(KT8, 1)
            load_x(1)
            nc.sync.dma_start(out=wv[:], in_=wv_d[:])
            nc.sync.dma_start(out=bv[:], in_=bv_d[:])
            nc.sync.dma_start(out=id_f32[:], in_=idf32_d[:])
            nc.vector.tensor_copy(bv_v[:], bv[:])

            # zero pads: V pad cols + ones cols (K/Q pads zeroed per chunk)
            v8_grp = Vt8.rearrange("p (n s) -> p n s", s=VG)
            nc.vector.memset(v8_grp[:, :, H:VG], 0.0)
            nc.vector.memset(v8_grp[:, :, H:G], 1.0)
            vb_grp = Vtb.rearrange("p (n s) -> p n s", s=G)
            nc.vector.memset(vb_grp[:, :, H:G], 1.0)

            # ---------------- projection of local chunk c ----------------
            def project_chunk(c):
                xT = spool.tile([128, 8 * KC], bf16, tag="xT")  # (db, row)
                for p in range(4):  # 128-row piece
                    tp = prj.tile([128, 1024], bf16, tag="prj")
                    for db in range(8):
                        nc.tensor.transpose(
                            tp[:, db * 128:(db + 1) * 128],
                            x_pieces[c][p][:, db * 128:(db + 1) * 128],
                            id_bf[:],
                        )
                    nc.vector.tensor_copy(
                        xT.rearrange("q (db r) -> q db r", r=KC)[:, :, p * 128:(p + 1) * 128],
                        tp.rearrange("q (db r) -> q db r", r=128),
                    )
                # stacked Q|K projection: out partitions 0:64 Q^T, 64:128 K^T
                ps_qk_t = prj.tile([128, KC], fp32, tag="prj")
                ps_qk = ps_qk_t[:]
                for db in range(8):
                    nc.tensor.matmul(
                        ps_qk[:],
                        wqk[:, db * 128:(db + 1) * 128],
                        xT[:, db * KC:(db + 1) * KC],
                        start=(db == 0), stop=(db == 7),
                    )
                # evac: h 0:64 on partitions 0:64 of the data cols
                kt_st = spool.tile([64, KC], f8, tag="kst")
                nc.vector.tensor_scalar_add(
                    kt_st[:], ps_qk[64:128, :], bqk[64:128, :])
                nc.vector.tensor_scalar_add(
                    QT8[0:64, 2 * c * KC:(2 * c + 1) * KC], ps_qk[0:64, :], bqk[0:64, :])

                # V projection (direct [k,h] layout), all 4 k-blocks in one bank
                ps_v_t = prj.tile([128, NKB * H], fp32, tag="prj")
                ps_v = ps_v_t[:]
                for kb in range(NKB):
                    for db in range(8):
                        nc.tensor.matmul(
                            ps_v[:, kb * H:(kb + 1) * H],
                            xT[:, db * KC + kb * 128: db * KC + (kb + 1) * 128],
                            wv[:, db * H:(db + 1) * H],
                            start=(db == 0), stop=(db == 7),
                        )
                vt_st = spool.tile([128, NKB * G], bf16, tag="vst")
                nc.vector.memset(
                    vt_st.rearrange("p (n s) -> p n s", s=G)[:, :, H:G], 1.0)
                nc.vector.tensor_add(
                    vt_st.rearrange("p (n s) -> p n s", s=G)[:, :, 0:H],
                    ps_v[:], bv_v[:],
                )
                return kt_st, vt_st

            def exchange_chunk(c, kt_st, vt_st):
                nc.sync.dma_start(
                    out=kv_out[c:c + 1, 0:KPKT].rearrange("o (h s) -> (o h) s", s=KC),
                    in_=kt_st.bitcast(u8),
                )
                nc.sync.dma_start(
                    out=kv_out[c:c + 1, KPKT:].rearrange("o (k g) -> (o k) g", g=NKB * G * 2),
                    in_=vt_st.bitcast(u8),
                )
                nc.gpsimd.collective_compute(
                    "AllGather",
                    mybir.AluOpType.bypass,
                    replica_groups=[[0, 4], [1, 5], [2, 6], [3, 7]],
                    ins=[kv_out[c:c + 1, :]],
                    outs=[kv_alls[c][:]],
                )
            def unpack_chunk(c):
                for r in range(2):
                    g = 2 * c + r  # global chunk index
                    nc.sync.dma_start(
                        out=KT8[0:64, 2 * g * KC:(2 * g + 1) * KC].bitcast(u8),
                        in_=kv_alls[c][r, 0:KPKT].rearrange("(h s) -> h s", s=KC),
                    )
                    nc.sync.dma_start(
                        out=Vtb[:, g * NKB * G:(g + 1) * NKB * G].bitcast(u8),
                        in_=kv_alls[c][r, KPKT:].rearrange("(k g) -> k g", g=NKB * G * 2),
                    )

            def vt8_copies(c):
                # deferred off the collective path so gathers flow back-to-back
                for r in range(2):
                    g = 2 * c + r
                    nc.gpsimd.tensor_copy(
                        Vt8.rearrange("p (n s) -> p n s", s=VG)
                           [:, g * NKB:(g + 1) * NKB, 0:G],
                        Vtb.rearrange("p (n s) -> p n s", s=G)
                           [:, g * NKB:(g + 1) * NKB, :],
                    )

            # ---------------- attention tile i ----------------
            KT8v = KT8.rearrange("p (g two s) -> p g two s", two=2, s=KC)
            QT8v = QT8.rearrange("p (g two s) -> p g two s", two=2, s=KC)

            def attention_slots(i):
                nslot = 2 * i + 2
                oT = psO.tile([128, QT], fp32, tag="oT")
                for j in range(nslot):
                    masked = j >= nslot - 2
                    if masked:
                        pX = xbpool.tile([128, NKB * KC], bf16, tag="pXb")
                    else:
                        pX = xppool.tile([128, NKB * KC], f8, tag="pX8")
                    for h2 in range(2):
                        sT = psS.tile([128, 2 * KC], fp32, tag="sT")
                        for kk in range(2):
                            kb = 2 * h2 + kk
                            nc.tensor.matmul(
                                sT[:, kk * KC:(kk + 1) * KC],
                                KT8v[:, j, :, kb * 128:(kb + 1) * 128],
                                QT8v[:, i],
                                start=True, stop=True, perf_mode=DR,
                            )
                        nc.scalar.activation(
                            pX[:, h2 * 2 * KC:(h2 + 1) * 2 * KC], sT[:], AF.Exp,
                            scale=1.0 / math.sqrt(H),
                        )
                        if masked:
                            mk = maskA if j == nslot - 2 else maskB
                            nc.vector.tensor_mul(
                                pX[:, h2 * 2 * KC:(h2 + 1) * 2 * KC],
                                pX[:, h2 * 2 * KC:(h2 + 1) * 2 * KC],
                                mk[:, h2 * 2 * KC:(h2 + 1) * 2 * KC],
                            )
                            for kk in range(2):
                                kb = 2 * h2 + kk
                                nc.tensor.matmul(
                                    oT[0:G, :],
                                    Vtb[:, (j * NKB + kb) * G:(j * NKB + kb + 1) * G],
                                    pX[:, kb * KC:(kb + 1) * KC],
                                    start=(j == 0 and kb == 0),
                                    stop=(j == nslot - 1 and kb == NKB - 1),
                                    skip_group_check=True,
                                )
                        else:
                            nc.tensor.matmul(
                                oT[0:VG, :],
                                Vt8[:, (j * NKB + 2 * h2) * VG:
                                       (j * NKB + 2 * h2 + 2) * VG]
                                    .rearrange("p (two g) -> p two g", two=2),
                                pX[:, h2 * 2 * KC:(h2 + 1) * 2 * KC]
                                    .rearrange("p (two s) -> p two s", two=2),
                                start=(j == 0 and h2 == 0),
                                stop=False, perf_mode=DR,
                                skip_group_check=True,
                            )
                return oT

            def attention_finA(i, oT):
                oT_sb = fpool.tile([128, QT], fp32, tag="oTsb")
                nc.vector.tensor_copy(oT_sb[0:G, :], oT[0:G, :])
                return oT_sb

            def attention_finB(i, oT_sb):
                po_t = prj.tile([128, 1024], bf16, tag="prj")
                po = po_t.bitcast(fp32)
                for t in range(NKB):
                    nc.tensor.transpose(
                        po[:, t * G:(t + 1) * G],
                        oT_sb[0:G, t * 128:(t + 1) * 128], id_f32[0:G, 0:G],
                    )
                rec = fpool.tile([128, NKB], fp32, tag="rec")
                nc.vector.reciprocal(
                    rec[:], po[:, 0:NKB * G].rearrange("p (t s) -> p t s", s=G)[:, :, H:G])
                y_sb = fpool.tile([128, NKB * H], fp32, tag="ysb")
                for t in range(NKB):
                    nc.vector.tensor_scalar_mul(
                        y_sb[:, t * H:(t + 1) * H], po[:, t * G: t * G + H],
                        rec[:, t:t + 1],
                    )
                nc.sync.dma_start(
                    out=y_d[i * QT:(i + 1) * QT, :].rearrange("(t p) h -> p t h", p=128),
                    in_=y_sb.rearrange("p (t h) -> p t h", h=H),
                )

            # ---------------- schedule ----------------
            st0 = project_chunk(0); exchange_chunk(0, *st0)
            unpack_chunk(0)
            nc.sync.dma_start(out=maskA[:], in_=maskA_d[:])
            kq_zero(KT8, 2); kq_zero(KT8, 3); kq_zero(QT8, 1)
            st1 = project_chunk(1); exchange_chunk(1, *st1)
            unpack_chunk(1)
            nc.sync.dma_start(out=maskB[:], in_=maskB_d[:])
            load_x(2)
            kq_zero(KT8, 4); kq_zero(KT8, 5); kq_zero(QT8, 2)
            vt8_copies(0)
            ob0 = attention_finA(0, attention_slots(0))
            st2 = project_chunk(2); exchange_chunk(2, *st2)
            unpack_chunk(2)
            load_x(3)
            kq_zero(KT8, 6); kq_zero(KT8, 7); kq_zero(QT8, 3)
            vt8_copies(1)
            ob1 = attention_finA(1, attention_slots(1))
            attention_finB(0, ob0)
            st3 = project_chunk(3); exchange_chunk(3, *st3)
            unpack_chunk(3)
            vt8_copies(2)
            ob2 = attention_finA(2, attention_slots(2))
            attention_finB(1, ob1)
            vt8_copies(3)
            ob3 = attention_finA(3, attention_slots(3))
            attention_finB(2, ob2)
            attention_finB(3, ob3)

    nc.compile()
    return nc


def _masks_for(role: int):
    # tri[kb][p, f] = 1.0 where f >= kb*128 + p  (keep q >= k in diag chunk)
    p = np.arange(128)[:, None]
    f = np.arange(KC)[None, :]
    tri = np.concatenate(
        [(f >= kb * 128 + p).astype(np.float32) for kb in range(NKB)], axis=1
    )
    ones = np.ones((128, NKB * KC), dtype=np.float32)
    zero = np.zeros((128, NKB * KC), dtype=np.float32)
    maskA = tri if role == 0 else ones
    maskB = zero if role == 0 else tri
    return (np.ascontiguousarray(maskA).astype(ml_dtypes.bfloat16),
            np.ascontiguousarray(maskB).astype(ml_dtypes.bfloat16))


def kernel(x, Wq_w, Wq_b, Wk_w, Wk_b, Wv_w, Wv_b):
    global _compiled, LAST_RESULT
    from concourse.bass_utils import run_bass_kernel_spmd

    x = np.asarray(x, dtype=np.float32)
    wqk_dm = np.concatenate([np.asarray(Wq_w), np.asarray(Wk_w)], axis=1)
    wqk = np.ascontiguousarray(
        wqk_dm.reshape(8, 128, 128).transpose(1, 0, 2).reshape(128, 8 * 128)
    ).astype(ml_dtypes.bfloat16)
    bqk = np.concatenate([np.asarray(Wq_b), np.asarray(Wk_b)])[:, None].astype(np.float32)
    wv = np.ascontiguousarray(
        np.asarray(Wv_w, dtype=np.float32).reshape(8, 128, H)
        .transpose(1, 0, 2).reshape(128, 8 * H)
    ).astype(ml_dtypes.bfloat16)
    bv = np.ascontiguousarray(np.tile(
        np.broadcast_to(np.asarray(Wv_b, dtype=np.float32)[None, :], (128, H)),
        (1, NKB)))

    if _compiled is None:
        _compiled = _build()
    nc = _compiled

    in_maps = []
    for c in range(8):
        b, role = c % 4, c // 4
        mA, mB = _masks_for(role)
        x_own = np.ascontiguousarray(
            x[b].reshape(NCHUNK, KC, D)[role::2].reshape(NT * KC, D)
        )
        in_maps.append({
            "x_kv": x_own,
            "wqk": wqk, "wv": wv, "bqk": bqk, "bv": bv,
            "maskA": mA, "maskB": mB,
            "zeros": np.zeros((128, 8 * KC), dtype=np.uint8),
            "idbf": np.eye(128, dtype=ml_dtypes.bfloat16),
            "idf32": np.eye(128, dtype=np.float32),
        })
    kw = {}
    if TRACE:
        kw = dict(trace=True, trace_cores=list(range(8)))
    res = run_bass_kernel_spmd(nc, in_maps, core_ids=list(range(8)), **kw)
    LAST_RESULT = res

    out = np.empty((B, S, H), dtype=np.float32)
    for c in range(8):
        b, role = c % 4, c // 4
        y = res.results[c]["y"]
        for i in range(NT):
            g = 2 * i + role
            out[b, g * QT:(g + 1) * QT, :] = y[i * QT:(i + 1) * QT, :]
    return out
